# revision 25
# baseline (speedup 1.0000x reference)
"""GaussianKernel (KAN-style RBF layer) Trainium2 Bass kernel, V2.

reference:
    h = (grid_max - grid_min) / (num_grids - 1)        # 4/7
    basis = exp(-((x[..., None] - grid) / h) ** 2)     # [B, IN, G]
    out = basis.reshape(B, IN * G) @ spline_weight     # [B, OUT]

Shapes: x [16384, 512] f32, grid [8] f32, spline_weight [4096, 512] f32.
Data-parallel over 8 cores (2048 rows each).

Design notes (measured on TRN2):

* Matmul issue cadence is ~216ns (259ns when the part is clocked down)
  per 512-free-dim matmul regardless of dtype; DoubleRow fp8 covers 2
  contraction chunks per matmul, everything else is 1. Exec time is
  dominated by 16 x (32 - n_fp8_chunks/2) matmuls, so the fp8 chunk set
  is pushed to the error gate. Config A1: grids {0,1,6,7} fully fp8
  (centered) + one mixed DR pair (g2,ic0)+(g5,ic3) -> 23 mm/tile.

* Basis production avoids the 8-DERF ACT bottleneck (~80us) via the
  exact factorization p_g = f_0 * E^g (E = exp(2*delta*x/h^2)), with
  p_g = f_g * RG_g, RG_g = exp((g_g^2-g_0^2)/h^2):
    ACT (one table set, zero ACT-table reloads):
      u0 = Square(1.75x+3.5); p0 = Exp(-u0) = f_0
      E1 = Exp(3.5x); E2 = Exp(7x); E4 = Exp(14x)
    DVE chain (7 tensor_tensors, binary decomposition keeps Exp-table
      error accumulation <= 3 factors): p2=p0E2, p3=p2E1, p1=p0E1,
      p4=p0E4, p5=p4E1, p6=p4E2, p7=p6E1.
  bf16 chunks feed matmuls as p_g with RG folded into the weights.

* fp8 grids are mean-centered before quantization ((p/RG - mu) -> fp8
  affine casts, DVE tensor_scalar / ACT Copy split 2-2 for engine
  balance; all-DVE for the first two batch chunks where ACT is the
  critical path). The exact bias sum_mu_g*w8_g[i,o] is restored by the
  PSUM->SBUF copy (DVE tensor_tensor add with a broadcast bias tile).

* Weight-side fp8 rounding error is shrunk host-side for free:
  regression-compensation folds the part of delta_w predictable from
  same-row bf16-grid basis values into the bf16 weights (+ bias).
  This measured ~0.4e-2 of rel-err headroom. (OBS-style joint rounding
  was tried and HURT the max-err metric - disabled by default.)

* Scheduling: batch chunks of widths [256,384,512,512,384] (ramp keeps
  ACT ahead of the tensor engine from the start; consecutive chunks
  need <= 8 psum banks). Steady-state chunks interleave bf16/DR issue
  (measured ~1.5us: DR LDWEIGHTS overlaps bf16 weight streams) and use
  full-width ACT/DVE ops; early chunks use per-half ops for faster
  trickle. Out DMA flushes per 128-row tile.

Measured: ~107us exec (fast clock) / ~125us (downclocked part),
rel err 1.752e-2 (gate 2e-2), identical across runs.
"""

import os
from contextlib import ExitStack

import numpy as np

import concourse.bass as bass
import concourse.bacc as bacc
import concourse.mybir as mybir
import concourse.tile as tile

N_CORES = 8
BATCH = 16384
B_CORE = BATCH // N_CORES  # 2048
IN_F = 512
OUT_F = 512
G = 8
B_CHUNK = 512
N_BC = B_CORE // B_CHUNK   # 4
BC_WIDTHS = [128, 384, 512, 512, 512]
N_IC = 4                   # 128-row in-feature blocks

FP32 = mybir.dt.float32
BF16 = mybir.dt.bfloat16
F8 = mybir.dt.float8e4
SQ = mybir.ActivationFunctionType.Square
EXP = mybir.ActivationFunctionType.Exp
COPY = mybir.ActivationFunctionType.Copy
ALU = mybir.AluOpType
DR = mybir.MatmulPerfMode.DoubleRow

GRID_MIN, GRID_MAX = -2.0, 2.0
H = (GRID_MAX - GRID_MIN) / (G - 1)
AEXP = 2.0 * H / H**2  # 2*delta/h^2 = 3.5
GRIDS = np.linspace(GRID_MIN, GRID_MAX, G)

# fp8 config: list of DR pairs; each pair is ((g1, ic1), (g2, ic2)).
# Same-grid adjacent pairs ((g,2h),(g,2h+1)) cast into the grid q tile;
# mixed pairs get a dedicated packed tile.
def _halves(*grids):
    return [((g, 2 * h), (g, 2 * h + 1)) for g in grids for h in (0, 1)]

GK_CFG = os.environ.get("GK_CFG", "A1")
_CFGS = {
    "A": _halves(0, 1, 6, 7),
    "A1": _halves(0, 1, 6, 7) + [((2, 0), (5, 3))],
    "A2": _halves(0, 1, 6, 7) + [((2, 0), (5, 3)), ((2, 1), (5, 2))],
    "A3": _halves(0, 1, 6, 7) + [((2, 0), (5, 3)), ((2, 1), (5, 2)),
                                 ((2, 2), (5, 1))],
}
FP8P = _CFGS[GK_CFG]
FP8_CHUNKS = sorted({c for pr in FP8P for c in pr})
FP8_GRIDS = sorted({g for g, _ in FP8_CHUNKS})
N_DRP = len(FP8P)
_BF_GRID_ORDER = [2, 3, 4, 5, 0, 1, 6, 7]  # readiness order
BF_CHUNKS = [(g, ic) for g in _BF_GRID_ORDER for ic in range(N_IC)
             if (g, ic) not in FP8_CHUNKS]
N_BFC = len(BF_CHUNKS)
N_MM = N_BFC + N_DRP  # matmuls per 128-row tile

OUT_BF16 = os.environ.get("GK_OUT_BF16", "0") == "1"
OUT_DT = BF16 if OUT_BF16 else FP32
W_OPT = os.environ.get("GK_W_OPT", "1") == "1"
OBS = os.environ.get("GK_OBS", "0") == "1"
# fp8 affine casts on ACT for the first GK_ACT_CASTS grids (rest on DVE)
ACT_CASTS = int(os.environ.get("GK_ACT_CASTS", "2"))

# chain bookkeeping: p_g = f_g * RG[g], RG[g] = exp((g_g^2 - g_0^2)/h^2)
RG = np.exp((GRIDS**2 - GRIDS[0]**2) / H**2)
# analytic means under N(0,1)
MUS = (H / np.sqrt(H**2 + 2.0)) * np.exp(-GRIDS**2 / (H**2 + 2.0))


def gaussian_kernel(ctx: ExitStack, tc: tile.TileContext,
                    out_ap: bass.AP, xt_ap: bass.AP,
                    wb_ap, w8_ap, bias_ap):
    nc = tc.nc

    const_pool = ctx.enter_context(tc.tile_pool(name="const", bufs=1))
    w_pool = ctx.enter_context(tc.tile_pool(name="w", bufs=1))
    x_pool = ctx.enter_context(tc.tile_pool(name="x", bufs=2))
    u_pool = ctx.enter_context(tc.tile_pool(name="u", bufs=1))
    p_pool = ctx.enter_context(tc.tile_pool(name="p", bufs=2))
    q_pool = ctx.enter_context(tc.tile_pool(name="q", bufs=2))
    out_pool = ctx.enter_context(tc.tile_pool(name="out_stage", bufs=4))
    psum_pool = ctx.enter_context(
        tc.tile_pool(name="psum_acc", bufs=8, space="PSUM"))

    b35 = const_pool.tile([128, 1], FP32, tag="b35")
    nc.gpsimd.memset(b35[:], 3.5)
    b05 = const_pool.tile([128, 1], FP32, tag="b05")
    nc.gpsimd.memset(b05[:], 0.5)
    bz = const_pool.tile([128, 1], FP32, tag="bz")
    nc.gpsimd.memset(bz[:], 0.0)

    # warm the exp_and_others table (square/exp/copy all live there)
    warm = const_pool.tile([128, 1], BF16, tag="warm")
    nc.scalar.activation(warm[:], bz[:], EXP, bias=bz[:], scale=1.0)

    # ---- weights: resident in SBUF, streamed on the ACT HWDGE queue ----
    # piece order matches matmul issue order (g3 chunks first)
    wb_sb = w_pool.tile([128, N_BFC, OUT_F], BF16, tag="wb")
    wb_src = wb_ap.rearrange("p (c o) -> p c o", c=N_BFC, o=OUT_F)
    nc.scalar.dma_start(wb_sb[:, 0:4, :], wb_src[:, 0:4, :])
    w8_sb = None
    if N_DRP:
        w8_sb = w_pool.tile([128, N_DRP, 2, OUT_F], F8, tag="w8")
        w8_src = w8_ap.rearrange("p (d t o) -> p d t o", d=N_DRP, t=2, o=OUT_F)
        c8 = min(2, N_DRP)
        nc.scalar.dma_start(w8_sb[:, 0:c8], w8_src[:, 0:c8])
    wb_mid = N_BFC // 2
    nc.sync.dma_start(wb_sb[:, 4:wb_mid, :], wb_src[:, 4:wb_mid, :])
    nc.scalar.dma_start(wb_sb[:, wb_mid:, :], wb_src[:, wb_mid:, :])
    if N_DRP and c8 < N_DRP:
        nc.scalar.dma_start(w8_sb[:, c8:], w8_src[:, c8:])
    # bias tile (needed only at the first flush) last on the queue
    bias_sb = const_pool.tile([128, OUT_F], FP32, tag="bias")
    nc.scalar.dma_start(bias_sb[:], bias_ap)

    def act_ops(tiles, sl):
        """ACT ops for ic slice sl, ordered for earliest bf16 readiness."""
        x_t, u_t, p, E1, E2, E4, q, qm = tiles
        nc.scalar.activation(u_t[:, 0, sl], x_t[:, sl], SQ,
                             bias=b35[:], scale=1.75)
        nc.scalar.activation(p[0][:, sl], u_t[:, 0, sl], EXP,
                             bias=bz[:], scale=-1.0)
        nc.scalar.activation(E2[:, sl], x_t[:, sl], EXP,
                             bias=bz[:], scale=2.0 * AEXP)
        nc.scalar.activation(E1[:, sl], x_t[:, sl], EXP,
                             bias=bz[:], scale=1.0 * AEXP)
        nc.scalar.activation(E4[:, sl], x_t[:, sl], EXP,
                             bias=bz[:], scale=4.0 * AEXP)

    def dve_chain(tiles, sl):
        x_t, u_t, p, E1, E2, E4, q, qm = tiles
        tt = lambda d, a, b: nc.vector.tensor_tensor(d[:, sl], a[:, sl],
                                                     b[:, sl], op=ALU.mult)
        tt(p[2], p[0], E2)
        tt(p[3], p[2], E1)
        tt(p[1], p[0], E1)
        tt(p[4], p[0], E4)
        tt(p[5], p[4], E1)
        tt(p[6], p[4], E2)
        tt(p[7], p[6], E1)

    def cast_one(dst_ap, src_ap, g, on_act=False):
        s = float(1.0 / RG[g])
        m = float(MUS[g])
        if on_act:
            nc.scalar.activation(dst_ap, src_ap, COPY, bias=-m, scale=s)
        else:
            nc.vector.tensor_scalar(dst_ap, src_ap, s, m,
                                    op0=ALU.mult, op1=ALU.subtract)

    def casts(tiles, half, act_ok=True):
        """fp8 affine casts for DR pairs whose chunks live in ic half."""
        x_t, u_t, p, E1, E2, E4, q, qm = tiles
        for d, pr in enumerate(FP8P):
            (g1, ic1), (g2, ic2) = pr
            if g1 == g2 and ic2 == ic1 + 1 and ic1 % 2 == 0:
                if ic1 // 2 != half:
                    continue
                on_act = act_ok and (FP8_GRIDS.index(g1) < ACT_CASTS)
                cast_one(q[g1][:, ic1:ic1 + 2], p[g1][:, ic1:ic1 + 2], g1,
                         on_act=on_act)
            else:
                for t, (g, ic) in enumerate(pr):
                    if ic // 2 != half:
                        continue
                    cast_one(qm[d][:, t:t + 1], p[g][:, ic:ic + 1], g)

    # uneven batch chunks: small first chunk -> ACT gets ahead early;
    # small last chunk -> short drain tail.
    n_off = 0
    BCS = []
    for w in BC_WIDTHS:
        BCS.append((n_off, w))
        n_off += w
    assert n_off == B_CORE

    def alloc_tiles(bci):
        off, w = BCS[bci]
        x_t = x_pool.tile([128, N_IC, w], BF16, tag="xt", name=f"xt{w}")
        for half in (0, 1):
            nc.sync.dma_start(
                x_t[:, 2 * half:2 * half + 2, :],
                xt_ap[:, 4 * off + 2 * half * w: 4 * off + (2 * half + 2) * w]
                .rearrange("p (ic b) -> p ic b", ic=2, b=w))
        u_t = u_pool.tile([128, 1, N_IC, w], FP32, tag="u", name=f"u{w}")
        p = {g: p_pool.tile([128, N_IC, w], BF16, tag=f"p{g}",
                            name=f"p{g}_{w}")
             for g in range(8)}
        E1 = p_pool.tile([128, N_IC, w], BF16, tag="E1", name=f"E1{w}")
        E2 = p_pool.tile([128, N_IC, w], BF16, tag="E2", name=f"E2{w}")
        E4 = p_pool.tile([128, N_IC, w], BF16, tag="E4", name=f"E4{w}")
        q = {g: q_pool.tile([128, N_IC, w], F8, tag=f"q{g}",
                            name=f"q{g}_{w}")
             for g in FP8_GRIDS}
        qm = {}
        for d, pr in enumerate(FP8P):
            (g1, ic1), (g2, ic2) = pr
            if not (g1 == g2 and ic2 == ic1 + 1 and ic1 % 2 == 0):
                qm[d] = q_pool.tile([128, 2, w], F8, tag=f"qm{d}",
                                    name=f"qm{d}_{w}")
        return (x_t, u_t, p, E1, E2, E4, q, qm)

    def prep(tiles, fine=True, casts_last=False):
        # early (fine) chunks keep ACT free for core ops: casts go to DVE
        if fine:
            for half in (0, 1):
                sl = slice(2 * half, 2 * half + 2)
                act_ops(tiles, sl)
                dve_chain(tiles, sl)
                if not casts_last:
                    casts(tiles, half, act_ok=False)
            if casts_last:
                casts(tiles, 0, act_ok=False)
                casts(tiles, 1, act_ok=False)
        else:
            act_ops(tiles, slice(0, 4))
            dve_chain(tiles, slice(0, 4))
            casts(tiles, 0)
            casts(tiles, 1)

    BF_ORDER = list(range(len(BF_CHUNKS)))  # already readiness-ordered

    def mm_and_flush(tiles, bci, interleave=False):
        """Per 128-row tile: one accumulation group then immediate flush."""
        off, w = BCS[bci]
        _, _, p, _, _, _, q, qm = tiles
        drs = list(enumerate(FP8P))
        ops = [("bf", c) for c in BF_ORDER] + [("dr", d) for d, _ in drs]
        if interleave:
            # alternate bf16 and DR so DR LDWEIGHTS overlaps bf16 streams
            bfs = [("bf", c) for c in BF_ORDER]
            dr2 = [("dr", d) for d, _ in drs]
            ops = []
            while bfs or dr2:
                if bfs:
                    ops.append(bfs.pop(0))
                if dr2:
                    ops.append(dr2.pop(0))
        n = len(ops)
        for bt in range(w // 128):
            bsl = slice(bt * 128, (bt + 1) * 128)
            pacc = psum_pool.tile([128, OUT_F], FP32, tag="pacc",
                                  name=f"pc{bci}_{bt}")
            for k, (kind, idx) in enumerate(ops):
                st, sp = (k == 0), (k == n - 1)
                if kind == "bf":
                    g, ic = BF_CHUNKS[idx]
                    nc.tensor.matmul(
                        pacc[:], p[g][:, ic:ic + 1, bsl],
                        wb_sb[:, idx:idx + 1, :], start=st, stop=sp)
                else:
                    (g1, ic1), _ = FP8P[idx]
                    if idx in qm:
                        lhsT = qm[idx][:, :, bsl]
                    else:
                        lhsT = q[g1][:, ic1:ic1 + 2, bsl]
                    nc.tensor.matmul(
                        pacc[:], lhsT, w8_sb[:, idx], start=st, stop=sp,
                        perf_mode=DR)
            os_t = out_pool.tile([128, OUT_F], OUT_DT, tag="os")
            nc.vector.tensor_tensor(os_t[:], pacc[:], bias_sb[:],
                                    op=ALU.add)
            nc.sync.dma_start(
                out_ap[off + bt * 128: off + (bt + 1) * 128, :], os_t[:])

    # ---- schedule ----
    tiles_cur = alloc_tiles(0)
    prep(tiles_cur, casts_last=True)
    tiles_next = alloc_tiles(1)
    prep(tiles_next)
    for bci in range(len(BCS)):
        mm_and_flush(tiles_cur, bci, interleave=(bci >= 2))
        tiles_cur = tiles_next
        if bci + 2 < len(BCS):
            tiles_next = alloc_tiles(bci + 2)
            prep(tiles_next, fine=False)


_CACHE = {}


def _build():
    key = (GK_CFG, OUT_BF16, ACT_CASTS)
    if key in _CACHE:
        return _CACHE[key]
    nc = bacc.Bacc("TRN2", target_bir_lowering=False, debug=False,
                   num_devices=N_CORES)
    xt_t = nc.dram_tensor("xt", [128, N_IC * B_CORE], BF16,
                          kind="ExternalInput")
    wb_t = nc.dram_tensor("wb", [128, N_BFC * OUT_F], BF16,
                          kind="ExternalInput")
    w8_t = (nc.dram_tensor("w8", [128, N_DRP * 2 * OUT_F], F8,
                           kind="ExternalInput") if N_DRP else None)
    bias_t = nc.dram_tensor("bias", [128, OUT_F], FP32, kind="ExternalInput")
    out_t = nc.dram_tensor("out", [B_CORE, OUT_F], OUT_DT,
                           kind="ExternalOutput")
    with tile.TileContext(nc) as tc:
        with ExitStack() as ctx:
            gaussian_kernel(ctx, tc, out_t.ap(), xt_t.ap(), wb_t.ap(),
                            w8_t.ap() if w8_t is not None else None,
                            bias_t.ap())
    nc.compile()
    _CACHE[key] = nc
    return nc


def _prep_weights(spline_weight: np.ndarray):
    """Host-side weight packing: bf16 chunks (with p-fold), fp8 DR pairs
    (joint optimal rounding in the residual-covariance metric +
    regression compensation through bf16 chunks), bias row."""
    import ml_dtypes

    F8NP = ml_dtypes.float8_e4m3
    w3 = np.ascontiguousarray(spline_weight, dtype=np.float64).reshape(
        IN_F, G, OUT_F)

    def rtn(a):
        return a.astype(np.float32).astype(F8NP).astype(np.float64)

    # centered covariance of basis values under N(0,1) (quadrature)
    xs, wq = np.polynomial.hermite_e.hermegauss(201)
    dens = wq / wq.sum()
    Fb = np.exp(-(((xs[:, None] - GRIDS) / H) ** 2))
    mub = dens @ Fb
    C = (Fb - mub).T @ (dens[:, None] * (Fb - mub))

    wq8 = {}       # (g, ic) -> [128, OUT] dequantized fp8 values
    wb_adj = {}    # (g, ic) -> [128, OUT] float64 adjustable bf16 weights
    for g, ic in BF_CHUNKS:
        wb_adj[(g, ic)] = w3[ic * 128:(ic + 1) * 128, g, :].copy()
    bias = np.zeros(OUT_F, dtype=np.float64)

    for ic in range(N_IC):
        rows = slice(ic * 128, (ic + 1) * 128)
        S = [g for g in range(G) if (g, ic) in FP8_CHUNKS]
        if not S:
            continue
        N = [g for g in range(G) if g not in S]
        W = w3[rows, :, :][:, S, :]               # [128, |S|, OUT]
        near = rtn(W)
        if W_OPT and OBS and len(S) > 1:
            alt = rtn(2.0 * W - near)
            Cnn = C[np.ix_(N, N)]
            Csn = C[np.ix_(S, N)]
            Ct = C[np.ix_(S, S)] - Csn @ np.linalg.solve(
                Cnn + 1e-12 * np.eye(len(N)), Csn.T)
            D = np.stack([near - W, alt - W], axis=0)  # [2,128,|S|,OUT]
            P = 1 << len(S)
            best_cost = None
            best_pat = np.zeros((128, OUT_F), dtype=np.int64)
            for pat in range(P):
                sel = [(pat >> j) & 1 for j in range(len(S))]
                Dp = np.stack([D[sel[j], :, j, :] for j in range(len(S))],
                              axis=1)              # [128,|S|,OUT]
                cost = np.einsum('gh,igo,iho->io', Ct, Dp, Dp,
                                 optimize=True)
                if best_cost is None:
                    best_cost = cost
                else:
                    m = cost < best_cost
                    best_cost = np.where(m, cost, best_cost)
                    best_pat = np.where(m, pat, best_pat)
            chosen = np.empty_like(near)
            for j in range(len(S)):
                bit = (best_pat >> j) & 1
                chosen[:, j, :] = np.where(bit == 1, alt[:, j, :],
                                           near[:, j, :])
        else:
            chosen = near
        for j, g in enumerate(S):
            wq8[(g, ic)] = chosen[:, j, :]

        if W_OPT and N:
            # regression compensation through same-ic bf16 chunks
            Cnn = C[np.ix_(N, N)]
            for j, g in enumerate(S):
                dw = wq8[(g, ic)] - w3[rows, g, :]
                A = np.linalg.solve(Cnn + 1e-12 * np.eye(len(N)), C[N, g])
                for n, a in zip(N, A):
                    if abs(a) < 1e-4 or (n, ic) not in wb_adj:
                        continue
                    wb_adj[(n, ic)] -= a * dw
                    bias += MUS[n] * (a * dw).sum(axis=0)

    wb_list = []
    for g, ic in BF_CHUNKS:
        wblk = wb_adj[(g, ic)] / RG[g]  # p_g = f_g*RG[g] => w~ = w/RG[g]
        wb_list.append(wblk.astype(np.float32))
    wb = np.stack(wb_list, axis=0)  # [N_BFC, 128, OUT]
    wb = np.ascontiguousarray(
        wb.transpose(1, 0, 2).reshape(128, N_BFC * OUT_F)
    ).astype(ml_dtypes.bfloat16)

    w8 = None
    if FP8P:
        blocks = []
        for pr in FP8P:
            pair_rows = []
            for g, ic in pr:
                blk = wq8[(g, ic)]  # [128, OUT]
                bias += MUS[g] * blk.sum(axis=0)
                pair_rows.append(blk)
            blocks.append(np.stack(pair_rows, axis=0))  # [2,128,OUT]
        w8s = np.stack(blocks, axis=0)  # [N_DRP, 2, 128, OUT]
        w8 = np.ascontiguousarray(
            w8s.transpose(2, 0, 1, 3).reshape(128, N_DRP * 2 * OUT_F)
        ).astype(F8NP)

    bias_full = np.ascontiguousarray(
        np.broadcast_to(bias.astype(np.float32), (128, OUT_F)))
    return wb, w8, bias_full


def kernel(x: np.ndarray, grid: np.ndarray, spline_weight: np.ndarray,
           _want_results=False, **_kw) -> np.ndarray:
    from concourse.bass_utils import run_bass_kernel_spmd
    import ml_dtypes

    nc = _build()
    wb, w8, bias_full = _prep_weights(spline_weight)

    # x pre-transposed per core (pure layout prep): [core, bc, p128, ic, b]
    x = np.ascontiguousarray(x, dtype=np.float32)
    xr = x.reshape(N_CORES, B_CORE, N_IC, 128)
    blocks = []
    off = 0
    for w in BC_WIDTHS:
        blk = xr[:, off:off + w].transpose(0, 3, 2, 1)  # [core,128,ic,w]
        blocks.append(np.ascontiguousarray(blk).reshape(N_CORES, 128, -1))
        off += w
    xt = np.ascontiguousarray(np.concatenate(blocks, axis=2)).astype(
        ml_dtypes.bfloat16)

    in_maps = []
    for i in range(N_CORES):
        m = {"xt": xt[i], "wb": wb, "bias": bias_full}
        if w8 is not None:
            m["w8"] = w8
        in_maps.append(m)
    res = run_bass_kernel_spmd(nc, in_maps, list(range(N_CORES)))
    out = np.concatenate(
        [np.asarray(res.results[i]["out"], dtype=np.float32)
         for i in range(N_CORES)], axis=0)
    if _want_results:
        return out, res
    return out
